# revision 5
# baseline (speedup 1.0000x reference)
"""Trainium2 Bass kernel for the LoRA-with-conditional-gating dense MLP.

Math (per batch element b):
    h        = LayerNorm(ctr_hidden[b]) * ln_gamma + ln_beta
    f        = h @ W_ctr.T + b_ctr                        # [CTR_F]
    sA       = f @ W_A_adapter.T                          # [R]
    sB       = f @ W_B_adapter.T                          # [D_OUT]
    a        = x[b] @ W_A.T                               # [S, R]
    out[b]   = (a * sA) @ W_B.T * sB * SCALING            # [S, D_OUT]

Both gates and the scaling fold into a tiny per-batch effective weight:
    W_eff.T[r, o] = SCALING * sA[r] * W_B[o, r] * sB[o]   # [R, D_OUT]
    out[b] = (x[b] @ W_A.T) @ W_eff.T

The scalar path (LayerNorm + three tiny matvecs, ~1.4 MFLOP total) is
computed on the host in float64; the device kernel does the two big
matmuls (2.7 GFLOP/core) and moves the x/out traffic.

v2 design (vs the fp32 v1 baseline at ~523 us):
  - bf16 operands end-to-end on device: PE streams at 1 cycle/row
    (fp32 is 4), DMA bytes halve (x 20 MiB in, out 20 MiB out percore).
  - x is transposed AND pre-tiled on the host into the [d-chunk, s]
    layout mm1 needs, so the 640 PE transpose instructions of v1
    disappear entirely (they were ~1/3 of v1's Tensor time).
  - Input DMA on the HWDGE (sync) queue, output DMA on the SWDGE
    (gpsimd) queue so the two 63 us streams overlap.
Predicted: Tensor ~68 us streaming + overheads, DMA ~63 us/queue.

Sharding: pure data-parallel over B=8 across the 8 NeuronCores (one
batch element per core, no collectives).

Accuracy: bf16 mantissa (8 bits) gives ~0.2% per-tensor rounding;
mm1+mm2 compound to ~0.5% relative — well inside the 2e-2 gate.
PSUM accumulation stays fp32.
"""

from contextlib import ExitStack

import numpy as np
import ml_dtypes

BF16 = ml_dtypes.bfloat16

# Problem shape (hardcoded per harness contract).
B, S = 8, 2048
D_IN = 5120
D_OUT = 5120
R = 64
CTR_H = 256
CTR_F = 128
ALPHA = 128.0
SCALING = ALPHA / R
LN_EPS = 1e-5

N_CORES = 8
P = 128                    # partitions
DCH = D_IN // P            # 40 d-chunks of 128
BS_BLK = 512               # bs rows per mm1 block (moving free dim)
N_BLK = S // BS_BLK        # 4
N_TILE = BS_BLK // P       # 4 row tiles of 128 per block
O_CH = 512                 # output chunk (one PSUM bank of fp32)
N_OCH = D_OUT // O_CH      # 10

_NC_CACHE = {}


def _build_nc(chain=1):
    """Build + compile the single-core SPMD Bass program (cached).

    chain > 1 wraps the whole body in a hardware For_i loop that re-runs
    it `chain` times — used by the timing harness to isolate device-exec
    time from host/RPC overhead. The graded path uses chain=1.
    """
    key = chain
    if key in _NC_CACHE:
        return _NC_CACHE[key]

    import concourse.bacc as bacc
    import concourse.mybir as mybir
    import concourse.tile as tile

    nc = bacc.Bacc("TRN2", target_bir_lowering=False, debug=False,
                   num_devices=N_CORES)
    f32 = mybir.dt.float32
    bf16 = mybir.dt.bfloat16

    # xp: host-pretransposed x. Row blk*128+p, col c*BS_BLK+s holds
    # x[blk*BS_BLK+s, c*128+p] — so a [128, DCH*BS_BLK] SBUF tile per
    # block has the d-contraction on partitions, one contiguous
    # 40 KiB DMA line per partition.
    xp_d = nc.dram_tensor("xp", [N_BLK * P, DCH * BS_BLK], bf16,
                          kind="ExternalInput")
    wa_d = nc.dram_tensor("wa_t", [P, DCH * R], bf16, kind="ExternalInput")
    weff_d = nc.dram_tensor("weff_t", [R, D_OUT], bf16, kind="ExternalInput")
    out_d = nc.dram_tensor("out", [S, D_OUT], bf16, kind="ExternalOutput")

    with tile.TileContext(nc) as tc, ExitStack() as ctx:
        const = ctx.enter_context(tc.tile_pool(name="const", bufs=1))
        x_pool = ctx.enter_context(tc.tile_pool(name="xp_sb", bufs=2))
        at_pool = ctx.enter_context(tc.tile_pool(name="at", bufs=2))
        out_pool = ctx.enter_context(tc.tile_pool(name="out_sb", bufs=3))
        ps_a = ctx.enter_context(tc.tile_pool(name="ps_a", bufs=2, space="PSUM"))
        ps_o = ctx.enter_context(tc.tile_pool(name="ps_o", bufs=3, space="PSUM"))

        wa_sb = const.tile([P, DCH * R], bf16)
        nc.sync.dma_start(out=wa_sb[:], in_=wa_d[:])
        weff_sb = const.tile([R, D_OUT], bf16)
        nc.sync.dma_start(out=weff_sb[:], in_=weff_d[:])

        loop_ctx = tc.For_i(0, chain, 1) if chain > 1 else None
        if loop_ctx is not None:
            ctx.enter_context(loop_ctx)

        # po->osb copy engines: rotate over DVE / Act (gpsimd cannot read
        # PSUM on HW).  The HW runs DMA transfers on parallel rings
        # (probed ~1.9 TB/s/core), so all DMA triggers sit on SP.
        cp_fns = [nc.vector.tensor_copy, nc.scalar.copy]
        cp_i = 0

        for blk in range(N_BLK):
            xb = x_pool.tile([P, DCH * BS_BLK], bf16, tag="xp_sb")
            nc.sync.dma_start(out=xb[:], in_=xp_d[blk * P:(blk + 1) * P, :])

            # mm1: aT[r, s] = sum_c waT_c.T @ xT_c, fp32 PSUM accumulation
            pa = ps_a.tile([R, BS_BLK], f32)
            for c in range(DCH):
                nc.tensor.matmul(pa[:], wa_sb[:, c * R:(c + 1) * R],
                                 xb[:, c * BS_BLK:(c + 1) * BS_BLK],
                                 start=(c == 0), stop=(c == DCH - 1))
            at = at_pool.tile([R, BS_BLK], bf16, tag="at")
            nc.vector.tensor_copy(at[:], pa[:])

            # mm2: out[s-tile, :] = at[:, s-tile].T @ W_eff.T.  Two 512-wide
            # matmuls land in one 2-bank PSUM tile; one 1024-wide copy
            # drains it (fewer instructions, 3-engine rotation keeps up
            # with PE's 213ns-per-chunk production rate).
            for t in range(N_TILE):
                row0 = blk * BS_BLK + t * P
                ats = at[:, t * P:(t + 1) * P]
                osb = out_pool.tile([P, D_OUT], bf16, tag="out_sb")
                for op in range(N_OCH // 2):
                    po = ps_o.tile([P, 2 * O_CH], f32, tag="po")
                    for h in range(2):
                        o = 2 * op + h
                        nc.tensor.matmul(po[:, h * O_CH:(h + 1) * O_CH],
                                         ats[:],
                                         weff_sb[:, o * O_CH:(o + 1) * O_CH],
                                         start=True, stop=True)
                    osl = slice(op * 2 * O_CH, (op + 1) * 2 * O_CH)
                    cp_fns[cp_i % 2](osb[:, osl], po[:])
                    cp_i += 1
                nc.sync.dma_start(out=out_d[row0: row0 + P, :], in_=osb[:])

    nc.compile()
    _NC_CACHE[key] = nc
    return nc


def _host_prep(ctr_hidden, ln_gamma, ln_beta, W_ctr, b_ctr,
               W_A_adapter, W_B_adapter, W_A, W_B):
    """Scalar path in float64; returns packed W_A.T and per-batch W_eff.T."""
    ch = np.asarray(ctr_hidden, dtype=np.float64)
    mu = ch.mean(axis=-1, keepdims=True)
    var = ((ch - mu) ** 2).mean(axis=-1, keepdims=True)
    h = (ch - mu) / np.sqrt(var + LN_EPS)
    h = h * np.asarray(ln_gamma, np.float64) + np.asarray(ln_beta, np.float64)
    f = h @ np.asarray(W_ctr, np.float64).T + np.asarray(b_ctr, np.float64)
    sA = f @ np.asarray(W_A_adapter, np.float64).T            # [B, R]
    sB = f @ np.asarray(W_B_adapter, np.float64).T            # [B, D_OUT]

    wbt = np.asarray(W_B, np.float64).T                       # [R, D_OUT]
    weff_t = (SCALING * sA[:, :, None] * wbt[None] * sB[:, None, :])
    weff_t = np.ascontiguousarray(weff_t.astype(BF16))        # [B, R, D_OUT]

    wa_t = np.asarray(W_A, np.float32).T.astype(BF16)         # [D_IN, R]
    wa_packed = np.ascontiguousarray(
        wa_t.reshape(DCH, P, R).transpose(1, 0, 2).reshape(P, DCH * R))
    return wa_packed, weff_t


def _pack_x(x_b):
    """[S, D_IN] fp32 -> [N_BLK*P, DCH*BS_BLK] bf16 pre-transposed tiles."""
    xb = np.asarray(x_b, np.float32).astype(BF16)
    xb = xb.reshape(N_BLK, BS_BLK, DCH, P).transpose(0, 3, 2, 1)
    return np.ascontiguousarray(xb.reshape(N_BLK * P, DCH * BS_BLK))


def _in_map(x_b, wa_packed, weff_b):
    return {"xp": _pack_x(x_b), "wa_t": wa_packed, "weff_t": weff_b}


def kernel(x, ctr_hidden, ln_gamma, ln_beta, W_ctr, b_ctr,
           W_A_adapter, W_B_adapter, W_A, W_B):
    from concourse import bass_utils

    x = np.asarray(x, dtype=np.float32)
    wa_packed, weff_t = _host_prep(ctr_hidden, ln_gamma, ln_beta, W_ctr, b_ctr,
                                   W_A_adapter, W_B_adapter, W_A, W_B)

    nc = _build_nc()
    in_maps = [_in_map(x[b], wa_packed, weff_t[b]) for b in range(B)]
    res = bass_utils.run_bass_kernel_spmd(nc, in_maps, list(range(N_CORES)))
    return np.stack([res.results[b]["out"] for b in range(B)]).astype(np.float32)


# revision 6
# speedup vs baseline: 1.0221x; 1.0221x over previous
"""Trainium2 Bass kernel for the LoRA-with-conditional-gating dense MLP.

Math (per batch element b):
    h        = LayerNorm(ctr_hidden[b]) * ln_gamma + ln_beta
    f        = h @ W_ctr.T + b_ctr                        # [CTR_F]
    sA       = f @ W_A_adapter.T                          # [R]
    sB       = f @ W_B_adapter.T                          # [D_OUT]
    a        = x[b] @ W_A.T                               # [S, R]
    out[b]   = (a * sA) @ W_B.T * sB * SCALING            # [S, D_OUT]

Both gates and the scaling fold into a tiny per-batch effective weight:
    W_eff.T[r, o] = SCALING * sA[r] * W_B[o, r] * sB[o]   # [R, D_OUT]
    out[b] = (x[b] @ W_A.T) @ W_eff.T

The scalar path (LayerNorm + three tiny matvecs, ~1.4 MFLOP total) is
computed on the host in float64; the device kernel does the two big
matmuls (2.7 GFLOP/core) and moves the x/out traffic.

v2 design (vs the fp32 v1 baseline at ~523 us):
  - bf16 operands end-to-end on device: PE streams at 1 cycle/row
    (fp32 is 4), DMA bytes halve (x 20 MiB in, out 20 MiB out percore).
  - x is transposed AND pre-tiled on the host into the [d-chunk, s]
    layout mm1 needs, so the 640 PE transpose instructions of v1
    disappear entirely (they were ~1/3 of v1's Tensor time).
  - Input DMA on the HWDGE (sync) queue, output DMA on the SWDGE
    (gpsimd) queue so the two 63 us streams overlap.
Predicted: Tensor ~68 us streaming + overheads, DMA ~63 us/queue.

Sharding: pure data-parallel over B=8 across the 8 NeuronCores (one
batch element per core, no collectives).

Accuracy: bf16 mantissa (8 bits) gives ~0.2% per-tensor rounding;
mm1+mm2 compound to ~0.5% relative — well inside the 2e-2 gate.
PSUM accumulation stays fp32.
"""

from contextlib import ExitStack

import numpy as np
import ml_dtypes

BF16 = ml_dtypes.bfloat16

# Problem shape (hardcoded per harness contract).
B, S = 8, 2048
D_IN = 5120
D_OUT = 5120
R = 64
CTR_H = 256
CTR_F = 128
ALPHA = 128.0
SCALING = ALPHA / R
LN_EPS = 1e-5

N_CORES = 8
P = 128                    # partitions
DCH = D_IN // P            # 40 d-chunks of 128
BS_BLK = 512               # bs rows per mm1 block (moving free dim)
N_BLK = S // BS_BLK        # 4
N_TILE = BS_BLK // P       # 4 row tiles of 128 per block
O_CH = 512                 # output chunk (one PSUM bank of fp32)
N_OCH = D_OUT // O_CH      # 10

_NC_CACHE = {}


def _build_nc(chain=1):
    """Build + compile the single-core SPMD Bass program (cached).

    chain > 1 wraps the whole body in a hardware For_i loop that re-runs
    it `chain` times — used by the timing harness to isolate device-exec
    time from host/RPC overhead. The graded path uses chain=1.
    """
    key = chain
    if key in _NC_CACHE:
        return _NC_CACHE[key]

    import concourse.bacc as bacc
    import concourse.mybir as mybir
    import concourse.tile as tile

    nc = bacc.Bacc("TRN2", target_bir_lowering=False, debug=False,
                   num_devices=N_CORES)
    f32 = mybir.dt.float32
    bf16 = mybir.dt.bfloat16

    # xp: host-pretransposed x. Row blk*128+p, col c*BS_BLK+s holds
    # x[blk*BS_BLK+s, c*128+p] — so a [128, DCH*BS_BLK] SBUF tile per
    # block has the d-contraction on partitions, one contiguous
    # 40 KiB DMA line per partition.
    xp_d = nc.dram_tensor("xp", [N_BLK * P, DCH * BS_BLK], bf16,
                          kind="ExternalInput")
    wa_d = nc.dram_tensor("wa_t", [P, DCH * R], bf16, kind="ExternalInput")
    weff_d = nc.dram_tensor("weff_t", [R, D_OUT], bf16, kind="ExternalInput")
    out_d = nc.dram_tensor("out", [S, D_OUT], bf16, kind="ExternalOutput")

    with tile.TileContext(nc) as tc, ExitStack() as ctx:
        const = ctx.enter_context(tc.tile_pool(name="const", bufs=1))
        x_pool = ctx.enter_context(tc.tile_pool(name="xp_sb", bufs=2))
        at_pool = ctx.enter_context(tc.tile_pool(name="at", bufs=2))
        out_pool = ctx.enter_context(tc.tile_pool(name="out_sb", bufs=3))
        ps_a = ctx.enter_context(tc.tile_pool(name="ps_a", bufs=2, space="PSUM"))
        ps_o = ctx.enter_context(tc.tile_pool(name="ps_o", bufs=3, space="PSUM"))

        wa_sb = const.tile([P, DCH * R], bf16)
        nc.sync.dma_start(out=wa_sb[:], in_=wa_d[:])
        weff_sb = const.tile([R, D_OUT], bf16)
        nc.sync.dma_start(out=weff_sb[:], in_=weff_d[:])

        loop_ctx = tc.For_i(0, chain, 1) if chain > 1 else None
        if loop_ctx is not None:
            ctx.enter_context(loop_ctx)

        # po->osb copy engines: rotate over DVE / Act (gpsimd cannot read
        # PSUM on HW).  Input DMA on the SP/HWDGE queue, output DMA on the
        # gpsimd/SWDGE queue: both queues are in-order, so mixing input and
        # output DMAs on one queue stalls block n+1's input behind block
        # n's compute-dependent output (measured 160us vs 130us).
        cp_fns = [nc.vector.tensor_copy, nc.scalar.copy]
        cp_i = 0

        def mm1(blk):
            xb = x_pool.tile([P, DCH * BS_BLK], bf16, tag="xp_sb")
            nc.sync.dma_start(out=xb[:], in_=xp_d[blk * P:(blk + 1) * P, :])
            # aT[r, s] = sum_c waT_c.T @ xT_c, fp32 PSUM accumulation
            pa = ps_a.tile([R, BS_BLK], f32)
            for c in range(DCH):
                nc.tensor.matmul(pa[:], wa_sb[:, c * R:(c + 1) * R],
                                 xb[:, c * BS_BLK:(c + 1) * BS_BLK],
                                 start=(c == 0), stop=(c == DCH - 1))
            at = at_pool.tile([R, BS_BLK], bf16, tag="at")
            nc.scalar.copy(at[:], pa[:])
            return at

        def mm2(blk, at):
            # out[s-tile, :] = at[:, s-tile].T @ W_eff.T.  Two 512-wide
            # matmuls land in one 2-bank PSUM tile; one 1024-wide copy
            # drains it.
            nonlocal cp_i
            for t in range(N_TILE):
                row0 = blk * BS_BLK + t * P
                ats = at[:, t * P:(t + 1) * P]
                osb = out_pool.tile([P, D_OUT], bf16, tag="out_sb")
                for op in range(N_OCH // 2):
                    po = ps_o.tile([P, 2 * O_CH], f32, tag="po")
                    for h in range(2):
                        o = 2 * op + h
                        nc.tensor.matmul(po[:, h * O_CH:(h + 1) * O_CH],
                                         ats[:],
                                         weff_sb[:, o * O_CH:(o + 1) * O_CH],
                                         start=True, stop=True)
                    osl = slice(op * 2 * O_CH, (op + 1) * 2 * O_CH)
                    cp_fns[cp_i % 2](osb[:, osl], po[:])
                    cp_i += 1
                nc.gpsimd.dma_start(out=out_d[row0: row0 + P, :], in_=osb[:])

        # Software pipeline: issue mm1(blk+1) before mm2(blk) so the PE
        # (in-order) never sits waiting on blk's PSUM->SBUF at-copy — it
        # streams mm1 of the next block during that latency.
        prev = None
        for blk in range(N_BLK):
            at = mm1(blk)
            if prev is not None:
                mm2(blk - 1, prev)
            prev = at
        mm2(N_BLK - 1, prev)

    nc.compile()
    _NC_CACHE[key] = nc
    return nc


def _host_prep(ctr_hidden, ln_gamma, ln_beta, W_ctr, b_ctr,
               W_A_adapter, W_B_adapter, W_A, W_B):
    """Scalar path in float64; returns packed W_A.T and per-batch W_eff.T."""
    ch = np.asarray(ctr_hidden, dtype=np.float64)
    mu = ch.mean(axis=-1, keepdims=True)
    var = ((ch - mu) ** 2).mean(axis=-1, keepdims=True)
    h = (ch - mu) / np.sqrt(var + LN_EPS)
    h = h * np.asarray(ln_gamma, np.float64) + np.asarray(ln_beta, np.float64)
    f = h @ np.asarray(W_ctr, np.float64).T + np.asarray(b_ctr, np.float64)
    sA = f @ np.asarray(W_A_adapter, np.float64).T            # [B, R]
    sB = f @ np.asarray(W_B_adapter, np.float64).T            # [B, D_OUT]

    wbt = np.asarray(W_B, np.float64).T                       # [R, D_OUT]
    weff_t = (SCALING * sA[:, :, None] * wbt[None] * sB[:, None, :])
    weff_t = np.ascontiguousarray(weff_t.astype(BF16))        # [B, R, D_OUT]

    wa_t = np.asarray(W_A, np.float32).T.astype(BF16)         # [D_IN, R]
    wa_packed = np.ascontiguousarray(
        wa_t.reshape(DCH, P, R).transpose(1, 0, 2).reshape(P, DCH * R))
    return wa_packed, weff_t


def _pack_x(x_b):
    """[S, D_IN] fp32 -> [N_BLK*P, DCH*BS_BLK] bf16 pre-transposed tiles."""
    xb = np.asarray(x_b, np.float32).astype(BF16)
    xb = xb.reshape(N_BLK, BS_BLK, DCH, P).transpose(0, 3, 2, 1)
    return np.ascontiguousarray(xb.reshape(N_BLK * P, DCH * BS_BLK))


def _in_map(x_b, wa_packed, weff_b):
    return {"xp": _pack_x(x_b), "wa_t": wa_packed, "weff_t": weff_b}


def kernel(x, ctr_hidden, ln_gamma, ln_beta, W_ctr, b_ctr,
           W_A_adapter, W_B_adapter, W_A, W_B):
    from concourse import bass_utils

    x = np.asarray(x, dtype=np.float32)
    wa_packed, weff_t = _host_prep(ctr_hidden, ln_gamma, ln_beta, W_ctr, b_ctr,
                                   W_A_adapter, W_B_adapter, W_A, W_B)

    nc = _build_nc()
    in_maps = [_in_map(x[b], wa_packed, weff_t[b]) for b in range(B)]
    res = bass_utils.run_bass_kernel_spmd(nc, in_maps, list(range(N_CORES)))
    return np.stack([res.results[b]["out"] for b in range(B)]).astype(np.float32)


# revision 7
# speedup vs baseline: 1.0353x; 1.0130x over previous
"""Trainium2 Bass kernel for the LoRA-with-conditional-gating dense MLP.

Math (per batch element b):
    h        = LayerNorm(ctr_hidden[b]) * ln_gamma + ln_beta
    f        = h @ W_ctr.T + b_ctr                        # [CTR_F]
    sA       = f @ W_A_adapter.T                          # [R]
    sB       = f @ W_B_adapter.T                          # [D_OUT]
    a        = x[b] @ W_A.T                               # [S, R]
    out[b]   = (a * sA) @ W_B.T * sB * SCALING            # [S, D_OUT]

Both gates and the scaling fold into a tiny per-batch effective weight:
    W_eff.T[r, o] = SCALING * sA[r] * W_B[o, r] * sB[o]   # [R, D_OUT]
    out[b] = (x[b] @ W_A.T) @ W_eff.T

The scalar path (LayerNorm + three tiny matvecs, ~1.4 MFLOP total) is
computed on the host in float64; the device kernel does the two big
matmuls (2.7 GFLOP/core) and moves the x/out traffic.

v2 design (vs the fp32 v1 baseline at ~523 us):
  - bf16 operands end-to-end on device: PE streams at 1 cycle/row
    (fp32 is 4), DMA bytes halve (x 20 MiB in, out 20 MiB out percore).
  - x is transposed AND pre-tiled on the host into the [d-chunk, s]
    layout mm1 needs, so the 640 PE transpose instructions of v1
    disappear entirely (they were ~1/3 of v1's Tensor time).
  - Input DMA on the HWDGE (sync) queue, output DMA on the SWDGE
    (gpsimd) queue so the two 63 us streams overlap.
Predicted: Tensor ~68 us streaming + overheads, DMA ~63 us/queue.

Sharding: pure data-parallel over B=8 across the 8 NeuronCores (one
batch element per core, no collectives).

Accuracy: bf16 mantissa (8 bits) gives ~0.2% per-tensor rounding;
mm1+mm2 compound to ~0.5% relative — well inside the 2e-2 gate.
PSUM accumulation stays fp32.
"""

from contextlib import ExitStack

import numpy as np
import ml_dtypes

BF16 = ml_dtypes.bfloat16

# Problem shape (hardcoded per harness contract).
B, S = 8, 2048
D_IN = 5120
D_OUT = 5120
R = 64
CTR_H = 256
CTR_F = 128
ALPHA = 128.0
SCALING = ALPHA / R
LN_EPS = 1e-5

N_CORES = 8
P = 128                    # partitions
DCH = D_IN // P            # 40 d-chunks of 128
BS_BLK = 512               # bs rows per mm1 block (moving free dim)
N_BLK = S // BS_BLK        # 4
N_TILE = BS_BLK // P       # 4 row tiles of 128 per block
O_CH = 512                 # output chunk (one PSUM bank of fp32)
N_OCH = D_OUT // O_CH      # 10

_NC_CACHE = {}


def _build_nc(chain=1):
    """Build + compile the single-core SPMD Bass program (cached).

    chain > 1 wraps the whole body in a hardware For_i loop that re-runs
    it `chain` times — used by the timing harness to isolate device-exec
    time from host/RPC overhead. The graded path uses chain=1.
    """
    key = chain
    if key in _NC_CACHE:
        return _NC_CACHE[key]

    import concourse.bacc as bacc
    import concourse.mybir as mybir
    import concourse.tile as tile

    nc = bacc.Bacc("TRN2", target_bir_lowering=False, debug=False,
                   num_devices=N_CORES)
    f32 = mybir.dt.float32
    bf16 = mybir.dt.bfloat16

    # xp: host-pretransposed x. Row blk*128+p, col c*BS_BLK+s holds
    # x[blk*BS_BLK+s, c*128+p] — so a [128, DCH*BS_BLK] SBUF tile per
    # block has the d-contraction on partitions, one contiguous
    # 40 KiB DMA line per partition.
    xp_d = nc.dram_tensor("xp", [N_BLK * P, DCH * BS_BLK], bf16,
                          kind="ExternalInput")
    wa_d = nc.dram_tensor("wa_t", [P, DCH * R], bf16, kind="ExternalInput")
    weff_d = nc.dram_tensor("weff_t", [R, D_OUT], bf16, kind="ExternalInput")
    out_d = nc.dram_tensor("out", [S, D_OUT], bf16, kind="ExternalOutput")

    with tile.TileContext(nc) as tc, ExitStack() as ctx:
        const = ctx.enter_context(tc.tile_pool(name="const", bufs=1))
        x_pool = ctx.enter_context(tc.tile_pool(name="xp_sb", bufs=2))
        at_pool = ctx.enter_context(tc.tile_pool(name="at", bufs=2))
        out_pool = ctx.enter_context(tc.tile_pool(name="out_sb", bufs=2))
        ps_a = ctx.enter_context(tc.tile_pool(name="ps_a", bufs=2, space="PSUM"))
        ps_o = ctx.enter_context(tc.tile_pool(name="ps_o", bufs=3, space="PSUM"))

        wa_sb = const.tile([P, DCH * R], bf16)
        nc.sync.dma_start(out=wa_sb[:], in_=wa_d[:])
        weff_sb = const.tile([R, D_OUT], bf16)
        nc.sync.dma_start(out=weff_sb[:], in_=weff_d[:])

        loop_ctx = tc.For_i(0, chain, 1) if chain > 1 else None
        if loop_ctx is not None:
            ctx.enter_context(loop_ctx)

        # po->osb copy engines: rotate over DVE / Act (gpsimd cannot read
        # PSUM on HW).  Input DMA on the SP/HWDGE queue, output DMA on the
        # gpsimd/SWDGE queue: both queues are in-order, so mixing input and
        # output DMAs on one queue stalls block n+1's input behind block
        # n's compute-dependent output (measured 160us vs 130us).
        cp_fns = [nc.scalar.copy, nc.vector.tensor_copy, nc.scalar.copy,
                  nc.vector.tensor_copy, nc.scalar.copy]
        cp_i = 0

        def mm1_ops(blk):
            xb = x_pool.tile([P, DCH * BS_BLK], bf16, tag="xp_sb")
            nc.sync.dma_start(out=xb[:], in_=xp_d[blk * P:(blk + 1) * P, :])
            pa = ps_a.tile([R, BS_BLK], f32)
            at = at_pool.tile([R, BS_BLK], bf16, tag="at")

            def op(c):
                nc.tensor.matmul(pa[:], wa_sb[:, c * R:(c + 1) * R],
                                 xb[:, c * BS_BLK:(c + 1) * BS_BLK],
                                 start=(c == 0), stop=(c == DCH - 1))
                if c == DCH - 1:
                    nc.vector.tensor_copy(at[:], pa[:])
            return [lambda c=c: op(c) for c in range(DCH)], at

        def mm2_ops(blk, at):
            # One block-wide osb tile; a single 3D-AP SWDGE DMA per block
            # (4 triggers/iter instead of 16 - SWDGE trigger cost is the
            # measured bottleneck).  Emitted as per-pair closures for
            # interleaving with the next block's mm1 chunks.
            osb = out_pool.tile([P, N_TILE * D_OUT], bf16, tag="out_sb")

            def pair(t, op_):
                nonlocal cp_i
                ats = at[:, t * P:(t + 1) * P]
                po = ps_o.tile([P, 2 * O_CH], f32, tag="po")
                for h in range(2):
                    o = 2 * op_ + h
                    nc.tensor.matmul(po[:, h * O_CH:(h + 1) * O_CH],
                                     ats[:],
                                     weff_sb[:, o * O_CH:(o + 1) * O_CH],
                                     start=True, stop=True)
                osl = slice(t * D_OUT + op_ * 2 * O_CH,
                            t * D_OUT + (op_ + 1) * 2 * O_CH)
                cp_fns[cp_i % 5](osb[:, osl], po[:])
                cp_i += 1
                if t == N_TILE - 1 and op_ == N_OCH // 2 - 1:
                    out_view = out_d[blk * BS_BLK:(blk + 1) * BS_BLK, :
                                     ].rearrange("(t p) o -> p t o", p=P)
                    nc.gpsimd.dma_start(
                        out=out_view,
                        in_=osb[:].rearrange("p (t o) -> p t o", t=N_TILE))
            return [lambda t=t, op_=op_: pair(t, op_)
                    for t in range(N_TILE) for op_ in range(N_OCH // 2)]

        def interleave(m1, m2, lead=6):
            # PE program order: `lead` mm1 chunks first (covers the at-copy
            # latency of the previous block), then 2 mm1 chunks per mm2
            # pair so the PSUM drains pace the PE without ever stalling it.
            if not m2:
                for f in m1:
                    f()
                return
            for f in m1[:lead]:
                f()
            i1, i2 = lead, 0
            while i1 < len(m1) or i2 < len(m2):
                for _ in range(2):
                    if i1 < len(m1):
                        m1[i1]()
                        i1 += 1
                if i2 < len(m2):
                    m2[i2]()
                    i2 += 1

        prev = None
        for blk in range(N_BLK):
            m1, at = mm1_ops(blk)
            m2 = mm2_ops(blk - 1, prev) if prev is not None else []
            interleave(m1, m2)
            prev = at
        for f in mm2_ops(N_BLK - 1, prev):
            f()

    nc.compile()
    _NC_CACHE[key] = nc
    return nc


def _host_prep(ctr_hidden, ln_gamma, ln_beta, W_ctr, b_ctr,
               W_A_adapter, W_B_adapter, W_A, W_B):
    """Scalar path in float64; returns packed W_A.T and per-batch W_eff.T."""
    ch = np.asarray(ctr_hidden, dtype=np.float64)
    mu = ch.mean(axis=-1, keepdims=True)
    var = ((ch - mu) ** 2).mean(axis=-1, keepdims=True)
    h = (ch - mu) / np.sqrt(var + LN_EPS)
    h = h * np.asarray(ln_gamma, np.float64) + np.asarray(ln_beta, np.float64)
    f = h @ np.asarray(W_ctr, np.float64).T + np.asarray(b_ctr, np.float64)
    sA = f @ np.asarray(W_A_adapter, np.float64).T            # [B, R]
    sB = f @ np.asarray(W_B_adapter, np.float64).T            # [B, D_OUT]

    wbt = np.asarray(W_B, np.float64).T                       # [R, D_OUT]
    weff_t = (SCALING * sA[:, :, None] * wbt[None] * sB[:, None, :])
    weff_t = np.ascontiguousarray(weff_t.astype(BF16))        # [B, R, D_OUT]

    wa_t = np.asarray(W_A, np.float32).T.astype(BF16)         # [D_IN, R]
    wa_packed = np.ascontiguousarray(
        wa_t.reshape(DCH, P, R).transpose(1, 0, 2).reshape(P, DCH * R))
    return wa_packed, weff_t


def _pack_x(x_b):
    """[S, D_IN] fp32 -> [N_BLK*P, DCH*BS_BLK] bf16 pre-transposed tiles."""
    xb = np.asarray(x_b, np.float32).astype(BF16)
    xb = xb.reshape(N_BLK, BS_BLK, DCH, P).transpose(0, 3, 2, 1)
    return np.ascontiguousarray(xb.reshape(N_BLK * P, DCH * BS_BLK))


def _in_map(x_b, wa_packed, weff_b):
    return {"xp": _pack_x(x_b), "wa_t": wa_packed, "weff_t": weff_b}


def kernel(x, ctr_hidden, ln_gamma, ln_beta, W_ctr, b_ctr,
           W_A_adapter, W_B_adapter, W_A, W_B):
    from concourse import bass_utils

    x = np.asarray(x, dtype=np.float32)
    wa_packed, weff_t = _host_prep(ctr_hidden, ln_gamma, ln_beta, W_ctr, b_ctr,
                                   W_A_adapter, W_B_adapter, W_A, W_B)

    nc = _build_nc()
    in_maps = [_in_map(x[b], wa_packed, weff_t[b]) for b in range(B)]
    res = bass_utils.run_bass_kernel_spmd(nc, in_maps, list(range(N_CORES)))
    return np.stack([res.results[b]["out"] for b in range(B)]).astype(np.float32)


# revision 10
# speedup vs baseline: 1.2814x; 1.2377x over previous
"""Trainium2 Bass kernel for the LoRA-with-conditional-gating dense MLP.

Math (per batch element b):
    h        = LayerNorm(ctr_hidden[b]) * ln_gamma + ln_beta
    f        = h @ W_ctr.T + b_ctr                        # [CTR_F]
    sA       = f @ W_A_adapter.T                          # [R]
    sB       = f @ W_B_adapter.T                          # [D_OUT]
    a        = x[b] @ W_A.T                               # [S, R]
    out[b]   = (a * sA) @ W_B.T * sB * SCALING            # [S, D_OUT]

Both gates and the scaling fold into a tiny per-batch effective weight:
    W_eff.T[r, o] = SCALING * sA[r] * W_B[o, r] * sB[o]   # [R, D_OUT]
    out[b] = (x[b] @ W_A.T) @ W_eff.T

The scalar path (LayerNorm + three tiny matvecs, ~1.4 MFLOP total) is
computed on the host in float64; the device kernel does the two big
matmuls (2.7 GFLOP/core) and moves the x/out traffic.

v2 design (vs the fp32 v1 baseline at ~523 us):
  - bf16 operands end-to-end on device: PE streams at 1 cycle/row
    (fp32 is 4), DMA bytes halve (x 20 MiB in, out 20 MiB out percore).
  - x is transposed AND pre-tiled on the host into the [d-chunk, s]
    layout mm1 needs, so the 640 PE transpose instructions of v1
    disappear entirely (they were ~1/3 of v1's Tensor time).
  - Input DMA on the HWDGE (sync) queue, output DMA on the SWDGE
    (gpsimd) queue so the two 63 us streams overlap.
Predicted: Tensor ~68 us streaming + overheads, DMA ~63 us/queue.

Sharding: pure data-parallel over B=8 across the 8 NeuronCores (one
batch element per core, no collectives).

Accuracy: bf16 mantissa (8 bits) gives ~0.2% per-tensor rounding;
mm1+mm2 compound to ~0.5% relative — well inside the 2e-2 gate.
PSUM accumulation stays fp32.
"""

from contextlib import ExitStack

import numpy as np
import ml_dtypes

BF16 = ml_dtypes.bfloat16

# Problem shape (hardcoded per harness contract).
B, S = 8, 2048
D_IN = 5120
D_OUT = 5120
R = 64
CTR_H = 256
CTR_F = 128
ALPHA = 128.0
SCALING = ALPHA / R
LN_EPS = 1e-5

N_CORES = 8
P = 128                    # partitions
DCH = D_IN // P            # 40 d-chunks of 128
BS_BLK = 512               # bs rows per mm1 block (moving free dim)
N_BLK = S // BS_BLK        # 4
N_TILE = BS_BLK // P       # 4 row tiles of 128 per block
O_CH = 512                 # output chunk (one PSUM bank of fp32)
N_OCH = D_OUT // O_CH      # 10

_NC_CACHE = {}


def _build_nc(chain=1):
    """Build + compile the single-core SPMD Bass program (cached).

    chain > 1 wraps the whole body in a hardware For_i loop that re-runs
    it `chain` times — used by the timing harness to isolate device-exec
    time from host/RPC overhead. The graded path uses chain=1.
    """
    key = chain
    if key in _NC_CACHE:
        return _NC_CACHE[key]

    import concourse.bacc as bacc
    import concourse.mybir as mybir
    import concourse.tile as tile

    nc = bacc.Bacc("TRN2", target_bir_lowering=False, debug=False,
                   num_devices=N_CORES)
    f32 = mybir.dt.float32
    bf16 = mybir.dt.bfloat16

    # xp: host-pretransposed x. Row blk*128+p, col c*BS_BLK+s holds
    # x[blk*BS_BLK+s, c*128+p] — so a [128, DCH*BS_BLK] SBUF tile per
    # block has the d-contraction on partitions, one contiguous
    # 40 KiB DMA line per partition.
    xp_d = nc.dram_tensor("xp", [N_BLK * P, DCH * BS_BLK], bf16,
                          kind="ExternalInput")
    wa_d = nc.dram_tensor("wa_t", [P, DCH * R], bf16, kind="ExternalInput")
    weff_d = nc.dram_tensor("weff_t", [R, D_OUT], bf16, kind="ExternalInput")
    out_d = nc.dram_tensor("out", [S, D_OUT], bf16, kind="ExternalOutput")

    with tile.TileContext(nc) as tc, ExitStack() as ctx:
        const = ctx.enter_context(tc.tile_pool(name="const", bufs=1))
        x_pool = ctx.enter_context(tc.tile_pool(name="xp_sb", bufs=2))
        at_pool = ctx.enter_context(tc.tile_pool(name="at", bufs=2))
        out_pool = ctx.enter_context(tc.tile_pool(name="out_sb", bufs=3))
        ps_a = ctx.enter_context(tc.tile_pool(name="ps_a", bufs=2, space="PSUM"))
        ps_o = ctx.enter_context(tc.tile_pool(name="ps_o", bufs=3, space="PSUM"))

        wa_sb = const.tile([P, DCH * R], bf16)
        nc.sync.dma_start(out=wa_sb[:], in_=wa_d[:])
        weff_sb = const.tile([R, D_OUT], bf16)
        nc.sync.dma_start(out=weff_sb[:], in_=weff_d[:])

        loop_ctx = tc.For_i(0, chain, 1) if chain > 1 else None
        if loop_ctx is not None:
            ctx.enter_context(loop_ctx)

        # po->osb copy engines: rotate over DVE / Act (gpsimd cannot read
        # PSUM on HW).  Input DMA on the SP/HWDGE queue, output DMA on the
        # gpsimd/SWDGE queue: both queues are in-order, so mixing input and
        # output DMAs on one queue stalls block n+1's input behind block
        # n's compute-dependent output (measured 160us vs 130us).
        cp_fns = [nc.scalar.copy, nc.vector.tensor_copy, nc.scalar.copy,
                  nc.vector.tensor_copy, nc.scalar.copy]
        cp_i = 0

        def mm1_ops(blk):
            xb = x_pool.tile([P, DCH * BS_BLK], bf16, tag="xp_sb")
            nc.sync.dma_start(out=xb[:], in_=xp_d[blk * P:(blk + 1) * P, :])
            pa = ps_a.tile([R, BS_BLK], f32)
            at = at_pool.tile([R, BS_BLK], bf16, tag="at")

            def op(c):
                nc.tensor.matmul(pa[:], wa_sb[:, c * R:(c + 1) * R],
                                 xb[:, c * BS_BLK:(c + 1) * BS_BLK],
                                 start=(c == 0), stop=(c == DCH - 1))
                if c == DCH - 1:
                    nc.vector.tensor_copy(at[:], pa[:])
            return [lambda c=c: op(c) for c in range(DCH)], at

        def mm2_ops(blk, at):
            # Half-block osb tiles [128, 2*D_OUT]: one 3D-AP SWDGE DMA per
            # 2 t-tiles (8 triggers/iter) with bufs=3 so copies never wait
            # on an in-flight output transfer.
            osbs = {}

            def pair(t, op_):
                nonlocal cp_i
                h2 = t // 2
                if h2 not in osbs:
                    osb_h = out_pool.tile([P, 2 * D_OUT], bf16,
                                          tag="out_sb", name=f"osb{h2}")
                    osbs[h2] = osb_h
                osb = osbs[h2]
                ats = at[:, t * P:(t + 1) * P]
                po = ps_o.tile([P, 2 * O_CH], f32, tag="po")
                for h in range(2):
                    o = 2 * op_ + h
                    nc.tensor.matmul(po[:, h * O_CH:(h + 1) * O_CH],
                                     ats[:],
                                     weff_sb[:, o * O_CH:(o + 1) * O_CH],
                                     start=True, stop=True)
                # split drain: DVE takes the low 512, Act the high 512 —
                # per-pair drain latency ~halves, so PSUM buffers free
                # faster and the PE never backs up on ps_o during bursts.
                base = (t % 2) * D_OUT + op_ * 2 * O_CH
                nc.vector.tensor_copy(osb[:, base:base + O_CH],
                                      po[:, 0:O_CH])
                nc.scalar.copy(osb[:, base + O_CH:base + 2 * O_CH],
                               po[:, O_CH:2 * O_CH])
                cp_i += 1
                if t % 2 == 1 and op_ == N_OCH // 2 - 1:
                    row0 = blk * BS_BLK + h2 * 2 * P
                    out_view = out_d[row0: row0 + 2 * P, :
                                     ].rearrange("(t p) o -> p t o", p=P)
                    nc.gpsimd.dma_start(
                        out=out_view,
                        in_=osb[:].rearrange("p (t o) -> p t o", t=2))
            return [lambda t=t, op_=op_: pair(t, op_)
                    for t in range(N_TILE) for op_ in range(N_OCH // 2)]

        def interleave(m1, m2, lead=6):
            # PE program order: `lead` mm1 chunks first (covers the at-copy
            # latency of the previous block), then 2 mm1 chunks per mm2
            # pair so the PSUM drains pace the PE without ever stalling it.
            if not m2:
                for f in m1:
                    f()
                return
            for f in m1[:lead]:
                f()
            i1, i2 = lead, 0
            while i1 < len(m1) or i2 < len(m2):
                for _ in range(2):
                    if i1 < len(m1):
                        m1[i1]()
                        i1 += 1
                if i2 < len(m2):
                    m2[i2]()
                    i2 += 1

        prev = None
        for blk in range(N_BLK):
            m1, at = mm1_ops(blk)
            m2 = mm2_ops(blk - 1, prev) if prev is not None else []
            interleave(m1, m2)
            prev = at
        for f in mm2_ops(N_BLK - 1, prev):
            f()

    nc.compile()
    _NC_CACHE[key] = nc
    return nc


def _host_prep(ctr_hidden, ln_gamma, ln_beta, W_ctr, b_ctr,
               W_A_adapter, W_B_adapter, W_A, W_B):
    """Scalar path in float64; returns packed W_A.T and per-batch W_eff.T."""
    ch = np.asarray(ctr_hidden, dtype=np.float64)
    mu = ch.mean(axis=-1, keepdims=True)
    var = ((ch - mu) ** 2).mean(axis=-1, keepdims=True)
    h = (ch - mu) / np.sqrt(var + LN_EPS)
    h = h * np.asarray(ln_gamma, np.float64) + np.asarray(ln_beta, np.float64)
    f = h @ np.asarray(W_ctr, np.float64).T + np.asarray(b_ctr, np.float64)
    sA = f @ np.asarray(W_A_adapter, np.float64).T            # [B, R]
    sB = f @ np.asarray(W_B_adapter, np.float64).T            # [B, D_OUT]

    wbt = np.asarray(W_B, np.float64).T                       # [R, D_OUT]
    weff_t = (SCALING * sA[:, :, None] * wbt[None] * sB[:, None, :])
    weff_t = np.ascontiguousarray(weff_t.astype(BF16))        # [B, R, D_OUT]

    wa_t = np.asarray(W_A, np.float32).T.astype(BF16)         # [D_IN, R]
    wa_packed = np.ascontiguousarray(
        wa_t.reshape(DCH, P, R).transpose(1, 0, 2).reshape(P, DCH * R))
    return wa_packed, weff_t


def _pack_x(x_b):
    """[S, D_IN] fp32 -> [N_BLK*P, DCH*BS_BLK] bf16 pre-transposed tiles."""
    xb = np.asarray(x_b, np.float32).astype(BF16)
    xb = xb.reshape(N_BLK, BS_BLK, DCH, P).transpose(0, 3, 2, 1)
    return np.ascontiguousarray(xb.reshape(N_BLK * P, DCH * BS_BLK))


def _in_map(x_b, wa_packed, weff_b):
    return {"xp": _pack_x(x_b), "wa_t": wa_packed, "weff_t": weff_b}


def kernel(x, ctr_hidden, ln_gamma, ln_beta, W_ctr, b_ctr,
           W_A_adapter, W_B_adapter, W_A, W_B):
    from concourse import bass_utils

    x = np.asarray(x, dtype=np.float32)
    wa_packed, weff_t = _host_prep(ctr_hidden, ln_gamma, ln_beta, W_ctr, b_ctr,
                                   W_A_adapter, W_B_adapter, W_A, W_B)

    nc = _build_nc()
    in_maps = [_in_map(x[b], wa_packed, weff_t[b]) for b in range(B)]
    res = bass_utils.run_bass_kernel_spmd(nc, in_maps, list(range(N_CORES)))
    return np.stack([res.results[b]["out"] for b in range(B)]).astype(np.float32)
